# revision 7
# baseline (speedup 1.0000x reference)
"""BertSum attention kernel v2 - per-pair pipelined, no DRAM spills.

Same math/sharding as v1 (see kernel.py docstring), but restructured:
dataT and mask stay resident in SBUF; for each head-pair p the kernel
projects kT/qT (and v for two pairs at a time), then immediately runs
attention for that pair. Tile pipelines pair p+1's projections (PE)
under pair p's attention epilogue (ACT/DVE), and weight slices stream
per pair so only ~184KB/partition of SBUF is needed.
"""

import numpy as np
from contextlib import ExitStack

import ml_dtypes

import concourse.bass as bass
import concourse.mybir as mybir
from concourse import bacc
from concourse.tile import TileContext
from concourse.bass_utils import run_bass_kernel_spmd

F32 = mybir.dt.float32
F32R = mybir.dt.float32r
BF16 = mybir.dt.bfloat16
AF = mybir.ActivationFunctionType
ALU = mybir.AluOpType

B, S, D = 4, 2048, 1024
H, DH = 16, 64
SQ = 1024
NP = 8

_CACHE = {}


def _build():
    nc = bacc.Bacc("TRN2", target_bir_lowering=False)

    dataT = nc.declare_dram_parameter("dataT", [D, S], F32R, isOutput=False)
    maskT = nc.declare_dram_parameter("maskT", [S, SQ], BF16, isOutput=False)
    wqT = nc.declare_dram_parameter("wqT", [D, D], F32R, isOutput=False)
    wkT = nc.declare_dram_parameter("wkT", [D, D], F32R, isOutput=False)
    wvT = nc.declare_dram_parameter("wvT", [D, D], F32R, isOutput=False)
    woT = nc.declare_dram_parameter("woT", [D, D], BF16, isOutput=False)
    bq2 = nc.declare_dram_parameter("bq2", [128, NP], F32, isOutput=False)
    bk2 = nc.declare_dram_parameter("bk2", [128, NP], F32, isOutput=False)
    boe = nc.declare_dram_parameter("boe", [1, D], BF16, isOutput=False)
    ones_r = nc.declare_dram_parameter("ones_r", [1, 128], F32R,
                                       isOutput=False)
    ones_b = nc.declare_dram_parameter("ones_b", [1, 128], BF16,
                                       isOutput=False)
    nc.declare_dram_parameter("chain", [1, 128], F32, isOutput=False)
    out = nc.declare_dram_parameter("out", [SQ, D], F32, isOutput=True)

    with ExitStack() as ctx:
        ctx.enter_context(nc.allow_low_precision(
            reason="fp32r matmul operand prep; accumulation stays f32"))
        tc = ctx.enter_context(TileContext(nc))
        const = ctx.enter_context(tc.tile_pool(name="const", bufs=1))
        dpool = ctx.enter_context(tc.tile_pool(name="data", bufs=1))
        ctxp = ctx.enter_context(tc.tile_pool(name="ctxT", bufs=1))

        ones = const.tile([1, 128], F32R)
        nc.sync.dma_start(out=ones, in_=ones_r[:, :])
        onesb = const.tile([1, 128], BF16)
        nc.sync.dma_start(out=onesb, in_=ones_b[:, :])
        boesb = const.tile([1, D], BF16)
        nc.sync.dma_start(out=boesb, in_=boe[:, :])
        bqsb = const.tile([128, NP], F32)
        nc.sync.dma_start(out=bqsb, in_=bq2[:, :])
        bksb = const.tile([128, NP], F32)
        nc.sync.dma_start(out=bksb, in_=bk2[:, :])

        dsb = []
        for i in range(8):
            t = dpool.tile([128, S], F32R, tag=f"d{i}", name=f"dsb{i}")
            nc.sync.dma_start(out=t, in_=dataT[i * 128:(i + 1) * 128, :])
            dsb.append(t)
        ctxT = [ctxp.tile([128, SQ], BF16, tag=f"ctx{p}", name=f"ctxT{p}")
                for p in range(NP)]

        with ExitStack() as actx:
            wkp = actx.enter_context(tc.tile_pool(name="wk", bufs=2))
            wqp = actx.enter_context(tc.tile_pool(name="wq", bufs=2))
            wvp = actx.enter_context(tc.tile_pool(name="wv", bufs=1))
            kpool = actx.enter_context(tc.tile_pool(name="kp", bufs=2))
            qpool = actx.enter_context(tc.tile_pool(name="qp", bufs=2))
            vpool = actx.enter_context(tc.tile_pool(name="vp", bufs=2))
            mpool = actx.enter_context(tc.tile_pool(name="mask", bufs=4))
            epool = actx.enter_context(tc.tile_pool(name="exp", bufs=3))
            rpool = actx.enter_context(tc.tile_pool(name="rec", bufs=1))
            cspool = actx.enter_context(tc.tile_pool(name="cs", bufs=1))
            pss = actx.enter_context(
                tc.tile_pool(name="pss", bufs=2, space="PSUM"))
            psc0 = actx.enter_context(
                tc.tile_pool(name="psc0", bufs=1, space="PSUM"))
            psc1 = actx.enter_context(
                tc.tile_pool(name="psc1", bufs=1, space="PSUM"))

            v_tiles = {}
            for p in range(NP):
                # ---- kT projection for pair p: [128ch, S] ----
                wk_sb = wkp.tile([128, 8, 128], F32R, tag="wk", name="wk_sb")
                for i in range(8):
                    nc.sync.dma_start(
                        out=wk_sb[:, i, :],
                        in_=wkT[i * 128:(i + 1) * 128,
                                p * 128:(p + 1) * 128])
                kT = kpool.tile([128, S], F32R, tag="k", name="kT")
                for sc in range(4):
                    ps = pss.tile([128, 512], F32, tag="ss", name="ps_k")
                    for i in range(8):
                        nc.tensor.matmul(
                            ps, wk_sb[:, i, :],
                            dsb[i][:, sc * 512:(sc + 1) * 512],
                            start=(i == 0), stop=(i == 7))
                    nc.vector.tensor_scalar_add(
                        kT[:, sc * 512:(sc + 1) * 512], ps, bksb[:, p:p + 1])

                # ---- qT projection for pair p: [128ch, SQ] ----
                wq_sb = wqp.tile([128, 8, 128], F32R, tag="wq", name="wq_sb")
                for i in range(8):
                    nc.sync.dma_start(
                        out=wq_sb[:, i, :],
                        in_=wqT[i * 128:(i + 1) * 128,
                                p * 128:(p + 1) * 128])
                qTt = qpool.tile([128, SQ], F32R, tag="q", name="qTt")
                for sc in range(2):
                    ps = pss.tile([128, 512], F32, tag="ss", name="ps_q")
                    for i in range(8):
                        nc.tensor.matmul(
                            ps, wq_sb[:, i, :],
                            dsb[i][:, sc * 512:(sc + 1) * 512],
                            start=(i == 0), stop=(i == 7))
                    nc.vector.tensor_scalar(
                        out=qTt[:, sc * 512:(sc + 1) * 512],
                        in0=ps, scalar1=0.125, scalar2=bqsb[:, p:p + 1],
                        op0=ALU.mult, op1=ALU.add)

                # ---- v projection for pairs (p, p+1), every other pair ----
                if p % 2 == 0:
                    wv_sb = wvp.tile([128, 8, 256], F32R, tag="wv",
                                     name="wv_sb")
                    for i in range(8):
                        nc.sync.dma_start(
                            out=wv_sb[:, i, :],
                            in_=wvT[i * 128:(i + 1) * 128,
                                    p * 128:(p + 2) * 128])
                    for j in range(2):
                        v_tiles[p + j] = vpool.tile(
                            [128, 16, 130], BF16, tag=f"v{j}", name=f"va{j}")
                    for st in range(16):
                        ps = pss.tile([128, 256], F32, tag="ss", name="ps_v")
                        for i in range(8):
                            nc.tensor.matmul(
                                ps, dsb[i][:, st * 128:(st + 1) * 128],
                                wv_sb[:, i, :],
                                start=(i == 0), stop=(i == 7))
                        for j in range(2):
                            va = v_tiles[p + j]
                            dst = va[:, st, :].rearrange(
                                "p (h c) -> p h c", c=65)
                            nc.vector.tensor_copy(
                                out=dst[:, :, 0:64],
                                in_=ps[:, j * 128:(j + 1) * 128].rearrange(
                                    "p (h c) -> p h c", c=64))
                            nc.vector.memset(dst[:, :, 64:65], 1.0)

                # ---- attention for pair p (heads interleaved per s) ----
                vt = v_tiles.pop(p)
                cps0 = psc0.tile([128, SQ], F32, tag="cps0", name="cps0")
                cps1 = psc1.tile([128, SQ], F32, tag="cps1", name="cps1")
                cpss = [cps0, cps1]
                for i in range(16):
                    mt = mpool.tile([128, SQ], BF16, tag="m", name="mt")
                    nc.sync.dma_start(
                        out=mt, in_=maskT[i * 128:(i + 1) * 128, :])
                    for h in range(2):
                        ss = pss.tile([128, SQ], F32, tag="ss", name="ss")
                        for qh in range(2):
                            nc.tensor.matmul(
                                ss[:, qh * 512:(qh + 1) * 512],
                                kT[h * 64:(h + 1) * 64,
                                   i * 128:(i + 1) * 128],
                                qTt[h * 64:(h + 1) * 64,
                                    qh * 512:(qh + 1) * 512],
                                start=True, stop=True)
                        et = epool.tile([128, SQ], BF16, tag="e", name="et")
                        nc.scalar.activation(out=et, in_=ss, func=AF.Exp)
                        nc.vector.tensor_mul(et, et, mt)
                        for qh in range(2):
                            nc.tensor.matmul(
                                cpss[h][0:65, qh * 512:(qh + 1) * 512],
                                vt[:, i, h * 65:(h + 1) * 65],
                                et[:, qh * 512:(qh + 1) * 512],
                                start=(i == 0), stop=(i == 15))
                for h in range(2):
                    rec32 = rpool.tile([1, SQ], F32, tag="r32", name="rec32")
                    nc.vector.reciprocal(rec32, cpss[h][64:65, :])
                    rec = rpool.tile([1, SQ], F32R, tag="r", name="rec")
                    nc.vector.tensor_scalar_mul(rec, rec32, 1.0)
                    bc = pss.tile([128, SQ], F32, tag="ss", name="bc")
                    for qh in range(2):
                        nc.tensor.matmul(
                            bc[0:64, qh * 512:(qh + 1) * 512],
                            ones[0:1, 0:64],
                            rec[0:1, qh * 512:(qh + 1) * 512],
                            start=True, stop=True)
                    cs = cspool.tile([64, SQ], F32, tag="cs", name="cs")
                    nc.vector.tensor_copy(cs, cpss[h][0:64, :])
                    nc.vector.tensor_mul(
                        ctxT[p][h * 64:(h + 1) * 64, :], cs, bc[0:64, :])

        # ---------------- output projection ------------------------------
        with ExitStack() as octx:
            wop = octx.enter_context(tc.tile_pool(name="wo", bufs=2))
            opool = octx.enter_context(tc.tile_pool(name="ost", bufs=3))
            pso = octx.enter_context(
                tc.tile_pool(name="pso", bufs=1, space="PSUM"))
            for dh in range(2):
                pso_t = [pso.tile([128, 512], F32, tag=f"o{qt}",
                                  name=f"pso{qt}") for qt in range(8)]
                for p in range(NP):
                    wo_sb = wop.tile([128, 512], BF16, tag="wo", name="wo_sb")
                    nc.sync.dma_start(
                        out=wo_sb,
                        in_=woT[p * 128:(p + 1) * 128,
                                dh * 512:(dh + 1) * 512])
                    for qt in range(8):
                        nc.tensor.matmul(
                            pso_t[qt], ctxT[p][:, qt * 128:(qt + 1) * 128],
                            wo_sb, start=(p == 0), stop=False)
                for qt in range(8):
                    nc.tensor.matmul(
                        pso_t[qt], onesb[0:1, 0:128],
                        boesb[0:1, dh * 512:(dh + 1) * 512],
                        start=False, stop=True)
                    ot = opool.tile([128, 512], F32, tag="ot", name="ot")
                    nc.vector.tensor_copy(ot, pso_t[qt])
                    nc.sync.dma_start(
                        out=out[qt * 128:(qt + 1) * 128,
                                dh * 512:(dh + 1) * 512],
                        in_=ot)

    nc.finalize()
    return nc


def _get_nc():
    if "nc" not in _CACHE:
        _CACHE["nc"] = _build()
    return _CACHE["nc"]


def _prep_inputs(data, mask, Wq, bq, Wk, bk, Wv, bv, Wo, bo):
    data = np.asarray(data, dtype=np.float32)
    mask = np.asarray(mask)
    WqT = np.ascontiguousarray(np.asarray(Wq, np.float32).T)
    WkT = np.ascontiguousarray(np.asarray(Wk, np.float32).T)
    WvT = np.ascontiguousarray(np.asarray(Wv, np.float32).T)
    WoT = np.ascontiguousarray(np.asarray(Wo, np.float32).T
                               .astype(ml_dtypes.bfloat16))
    bq2 = np.ascontiguousarray((np.asarray(bq, np.float32) / 8.0)
                               .reshape(NP, 128).T)
    bk2 = np.ascontiguousarray(np.asarray(bk, np.float32)
                               .reshape(NP, 128).T)
    boe = (np.asarray(bo, np.float32)
           + np.asarray(Wo, np.float32) @ np.asarray(bv, np.float32))
    boe = np.ascontiguousarray(boe.reshape(1, D)).astype(ml_dtypes.bfloat16)
    ones_r = np.ones((1, 128), np.float32)
    ones_b = np.ones((1, 128), ml_dtypes.bfloat16)

    in_maps = []
    for c in range(8):
        b, half = divmod(c, 2)
        q0 = half * SQ
        perm = np.concatenate(
            [np.arange(q0, q0 + SQ), np.arange((1 - half) * SQ,
                                               (1 - half) * SQ + SQ)])
        dT = np.ascontiguousarray(data[b].T[:, perm])
        keep = ~mask[b, q0:q0 + SQ, :]
        mT = np.ascontiguousarray(
            keep.T[perm, :].astype(ml_dtypes.bfloat16))
        in_maps.append({
            "dataT": dT, "maskT": mT,
            "wqT": WqT, "wkT": WkT, "wvT": WvT, "woT": WoT,
            "bq2": bq2, "bk2": bk2, "boe": boe,
            "ones_r": ones_r, "ones_b": ones_b,
            "chain": np.zeros((1, 128), np.float32),
        })
    return in_maps


def kernel(**inputs):
    in_maps = _prep_inputs(**inputs)
    nc = _get_nc()
    res = run_bass_kernel_spmd(nc, in_maps, list(range(8))).results
    out = np.empty((B, S, D), np.float32)
    for c in range(8):
        b, half = divmod(c, 2)
        out[b, half * SQ:(half + 1) * SQ, :] = res[c]["out"]
    return out
